# revision 2
# baseline (speedup 1.0000x reference)
"""Causal self-attention Trainium2 Bass kernel, v3 (fused stream, bf16
GEMMs, hi/lo-residual fp8 PV).

Problem: B=2, N=2048, D=1024, H=16 heads, DH=64 (fp32).
  kqv = einsum('bnd,hed->bhne', x, Wqkv) + bqkv   (chunk order k, q, v)
  scores = q @ k^T / 8, causal mask, softmax
  sa = attn @ v, concat heads, out = sa @ Wproj.T + bproj

Sharding (8 cores): data-parallel over B (2) x tensor-parallel over heads
(4 heads/core).  Host sums the 4 partials per batch and adds bproj_eff
(bproj + Wproj @ bv; softmax weights sum to 1, so the v-bias folds out).

Design (engine budgets per core: PE ~108us, ScalarE ~80us, DVE ~86us):
  - Single fused instruction stream: QKV GEMM slices emitted just-in-time
    between attention blocks so ScalarE (exp) starts ~10us in, and
    next-qb GEMMs fill the PE gap left by the last head's norm chain
    (prevents the HAM clock-gate from throttling at qb boundaries).
  - All GEMMs in bf16 (fp8 GEMMs measured 2-4e-2 error: v/kq fp8
    el-wise noise passes through softmax averaging unsuppressed).
  - V path: direct [n, e] GEMM; the PSUM result is split into fp8
    hi + residual-lo (v8 = fp8(v), r8 = fp8(v - v8), combined error
    ~0.07%) packed as the two DoubleRow subtiles of the PV lhsT, with
    denominator ones (hi) / zeros (lo) columns alongside.  PV runs as
    one DoubleRow matmul per m-tile with a stride-0-broadcast pt rhs:
    2x column rate, full precision, denominator for free.
  - Causal mask accumulated into score PSUM by identity-lhsT matmuls
    (-2^18 at masked positions); fully-masked column ranges of diagonal
    tiles are skipped in the scores/exp/PV column windows.
  - exp: scale 1/8, bias -2.5 (keeps pt = exp(s-2.5) under fp8 max 240;
    uniform shift cancels in normalization); pt in fp8 (error cancels
    between numerator and denominator to first order).
  - q bias-add uses tensor_scalar mult+add with 0/1 per-partition masks
    so the other head's partitions are zeroed in the same op.
  - proj bf16; output partials DMA'd as bf16, summed on host in fp32.
"""

import numpy as np
from contextlib import ExitStack

B, N, D, H = 2, 2048, 1024, 16
DH = 64
NH = 4                    # heads per core
DT = D // 128             # 8 d-tiles (contraction)
NBS = 512                 # n block size (query block)
NB = N // NBS             # 4 n blocks
MTS = 128                 # m tile size (key-axis tile)
MT = N // MTS             # 16 m tiles
KT = NH * DH // 128       # 2 proj contraction tiles (256 local d_in)
MASKVAL = -262144.0       # -2^18; *0.125 = -32768 post exp-scale
EXPSCALE = 0.125
EXPBIAS = -2.5            # keeps exp(s) below fp8 max; cancels in norm

_CACHE = {}


def _build_nc():
    import concourse.mybir as mybir
    import concourse.tile as tile
    from concourse import bacc

    f32 = mybir.dt.float32
    bf16 = mybir.dt.bfloat16
    fp8 = mybir.dt.float8e4
    EXP = mybir.ActivationFunctionType.Exp
    DR = mybir.MatmulPerfMode.DoubleRow
    MULT = mybir.AluOpType.mult
    ADD = mybir.AluOpType.add

    nc = bacc.Bacc("TRN2")
    xT_d = nc.dram_tensor("xT", [DT, 128, N], bf16, kind="ExternalInput")
    wkq_d = nc.dram_tensor("wkq", [4, DT, 128, 128], bf16,
                           kind="ExternalInput")
    wv_d = nc.dram_tensor("wv", [DT, 128, NH * DH], bf16,
                          kind="ExternalInput")
    bkq_d = nc.dram_tensor("bkq", [4, 128], f32, kind="ExternalInput")
    qm_d = nc.dram_tensor("qm", [3, 2, 128], f32, kind="ExternalInput")
    wpT_d = nc.dram_tensor("wpT", [NH * DH, D], bf16, kind="ExternalInput")
    mneg_d = nc.dram_tensor("mneg", [2, 128, 128], bf16, kind="ExternalInput")
    id_d = nc.dram_tensor("ident", [128, 128], bf16, kind="ExternalInput")
    out_d = nc.dram_tensor("outp", [N, D], bf16, kind="ExternalOutput")

    with tile.TileContext(nc) as tc, ExitStack() as ctx:
        const = ctx.enter_context(tc.tile_pool(name="const", bufs=1))

        # gpsimd SWDGE ring: consts + late x chunks; sync ring: early x.
        # Ordering matters: everything the first attention block needs
        # lands first on each ring.
        ident = const.tile([128, 128], bf16)
        nc.gpsimd.dma_start(out=ident, in_=id_d[:, :])
        wv16 = const.tile([128, DT, NH * DH], bf16)
        nc.gpsimd.dma_start(
            out=wv16, in_=wv_d.rearrange("t p e -> p t e"))
        wkq16 = const.tile([128, 4, DT, 128], bf16)
        for et in (0, 2):
            nc.gpsimd.dma_start(out=wkq16[:, et, :, :],
                                in_=wkq_d.rearrange("a t p e -> p a t e")
                                [:, et, :, :])
        bkq = const.tile([128, 4, 1], f32)
        nc.gpsimd.dma_start(
            out=bkq, in_=bkq_d.rearrange("a (p o) -> p a o", o=1))
        qm = const.tile([128, 3, 2, 1], f32)   # [j, {scale,bias_q01,bias_q23}]
        nc.gpsimd.dma_start(
            out=qm, in_=qm_d.rearrange("s j (p o) -> p s j o", o=1))
        mneg = const.tile([128, 2, 128], bf16)
        nc.gpsimd.dma_start(out=mneg, in_=mneg_d.rearrange("r p f -> p r f"))

        xT = const.tile([128, DT, N], bf16)
        xTr = xT_d.rearrange("t p n -> p t n")
        for t in range(6):
            nc.sync.dma_start(out=xT[:, t, :], in_=xTr[:, t, :])
        for t in range(6, 8):
            nc.gpsimd.dma_start(out=xT[:, t, :], in_=xTr[:, t, :])
        for et in (1, 3):
            nc.gpsimd.dma_start(out=wkq16[:, et, :, :],
                                in_=wkq_d.rearrange("a t p e -> p a t e")
                                [:, et, :, :])
        wpT = const.tile([128, KT, D], bf16)
        nc.gpsimd.dma_start(out=wpT, in_=wpT_d.rearrange(
            "(t p) f -> p t f", p=128))

        kqv = const.tile([128, 2, N], bf16)       # k01, k23 e-tiles
        qpad = [const.tile([128, N], bf16, name=f"qpad{h}")
                for h in range(NH)]
        # packed V: [mt, h, hl, {v,ones}, 64] fp8
        # hi = [v8 | 1.0], lo = [r8 | 0.0]
        vaug = const.tile([128, MT, NH, 2, 2, DH], fp8)
        nc.vector._memset_packed(
            vaug[:, :, :, 0, 1, :].bitcast(mybir.dt.uint16), 0x3838)
        nc.vector._memset_packed(
            vaug[:, :, :, 1, 1, :].bitcast(mybir.dt.uint16), 0)
        saT = const.tile([128, KT, N], bf16)      # sa^T, d_in on partitions
        ebias = const.tile([128, 1], f32)         # exp bias (see EXPBIAS)
        nc.vector.memset(ebias, EXPBIAS)

        with tc.tile_pool(name="sps", bufs=2, space="PSUM") as sps, \
             tc.tile_pool(name="sap", bufs=2, space="PSUM") as sapp, \
             tc.tile_pool(name="mix", bufs=2, space="PSUM") as mix, \
             tc.tile_pool(name="pts", bufs=8) as pts, \
             tc.tile_pool(name="ptb", bufs=4) as ptb, \
             tc.tile_pool(name="rrp", bufs=4) as rrp, \
             tc.tile_pool(name="ost", bufs=4) as ost:

            # HAM warmup: keep the PE busy while input DMAs land so the
            # clock gate releases the 1.2 GHz throttle before real work
            warm = mix.tile([128, 128], bf16, name="warm", tag="mixp",
                            bufs=2)
            with nc.allow_low_precision(reason="HAM warmup spin"):
                for _ in range(60):
                    nc.tensor.transpose(warm, ident, ident)
            nc.scalar.copy(saT[:, 0, 0:1], warm[:, 0:1])

            def v_gemm(nt):
                pv = mix.tile([128, 512], f32, tag="mixp", name=f"pv{nt}")
                for t in range(DT):
                    nc.tensor.matmul(
                        pv[:, 0:NH * DH],
                        lhsT=xT[:, t, nt * 128:(nt + 1) * 128],
                        rhs=wv16[:, t, :],
                        start=(t == 0), stop=(t == DT - 1))
                pvh = pv[:, 0:NH * DH].rearrange("p (h e) -> p h e", e=DH)
                nc.vector.tensor_copy(vaug[:, nt, :, 0, 0, :], pvh)
                nc.vector.tensor_sub(vaug[:, nt, :, 1, 0, :], pvh,
                                     vaug[:, nt, :, 0, 0, :])

            def kq_gemm(et, nb):
                # et: 0=k01 1=k23 2=q01 3=q23
                nbs = slice(nb * NBS, (nb + 1) * NBS)
                ps = mix.tile([128, NBS], f32, tag="mixp", name=f"kq{et}{nb}")
                for t in range(DT):
                    nc.tensor.matmul(
                        ps,
                        lhsT=wkq16[:, et, t, :],
                        rhs=xT[:, t, nbs],
                        start=(t == 0), stop=(t == DT - 1))
                if et >= 2:
                    # qpad[hh] = ps * mask01 + bias*mask01 (other head's
                    # partitions zeroed in the same op)
                    for j in range(2):
                        hh = 2 * (et - 2) + j
                        nc.vector.tensor_scalar(
                            out=qpad[hh][:, nbs],
                            in0=ps,
                            scalar1=qm[:, 0, j, :],
                            scalar2=qm[:, et - 1, j, :],
                            op0=MULT, op1=ADD)
                else:
                    nc.vector.tensor_scalar_add(
                        out=kqv[:, et, nbs],
                        in0=ps,
                        scalar1=bkq[:, et, :])

            def attn(qb, h):
                kt_tile = kqv[:, h // 2, :]
                qmv = qpad[h][:, qb * NBS:(qb + 1) * NBS]
                sap = sapp.tile([128, NBS], f32, name="sap")
                npair = 2 * qb + 2
                for mp in range(npair):
                    diag_a = (mp == 2 * qb)      # m-tiles 4qb, 4qb+1
                    diag_b = (mp == 2 * qb + 1)  # m-tiles 4qb+2, 4qb+3
                    sp = sps.tile([128, 2, NBS], f32, name="sp")
                    for j in range(2):
                        mt = 2 * mp + j
                        ktile = kt_tile[:, mt * MTS:(mt + 1) * MTS]
                        if diag_b:
                            nc.tensor.matmul(
                                sp[:, j, 256:512], lhsT=ktile,
                                rhs=qmv[:, 256:512],
                                start=True, stop=False)
                            if j == 0:   # r2: jagged at [256:384)
                                nc.tensor.matmul(
                                    sp[:, j, 256:384], lhsT=ident,
                                    rhs=mneg[:, 1, :],
                                    start=False, stop=True)
                            else:        # r3: full+jagged at [256:512)
                                nc.tensor.matmul(
                                    sp[:, j, 256:512], lhsT=ident,
                                    rhs=mneg.rearrange("p r f -> p (r f)"),
                                    start=False, stop=True)
                        elif diag_a:
                            if j == 0:   # r0: full cols, jagged [0:128)
                                nc.tensor.matmul(
                                    sp[:, j, :], lhsT=ktile, rhs=qmv,
                                    start=True, stop=False)
                                nc.tensor.matmul(
                                    sp[:, j, 0:128], lhsT=ident,
                                    rhs=mneg[:, 1, :],
                                    start=False, stop=True)
                            else:        # r1: cols [128:512), jag [128:256)
                                nc.tensor.matmul(
                                    sp[:, j, 128:512], lhsT=ktile,
                                    rhs=qmv[:, 128:512],
                                    start=True, stop=False)
                                nc.tensor.matmul(
                                    sp[:, j, 128:256], lhsT=ident,
                                    rhs=mneg[:, 1, :],
                                    start=False, stop=True)
                        else:
                            nc.tensor.matmul(
                                sp[:, j, :], lhsT=ktile, rhs=qmv,
                                start=True, stop=True)
                    if diag_b:
                        pt = ptb.tile([128, 2, 256], fp8, name="ptb")
                        nc.scalar.activation(pt, sp[:, :, 256:512], EXP,
                                             scale=EXPSCALE, bias=ebias)
                    elif diag_a:
                        pt = pts.tile([128, 2, NBS], fp8, name="pt")
                        # r1 cols [0:128) were never computed: exp only
                        # the valid windows (r0 full, r1 [128:512))
                        nc.scalar.activation(pt[:, 0, :], sp[:, 0, :], EXP,
                                             scale=EXPSCALE, bias=ebias)
                        nc.scalar.activation(pt[:, 1, 128:512],
                                             sp[:, 1, 128:512], EXP,
                                             scale=EXPSCALE, bias=ebias)
                    else:
                        pt = pts.tile([128, 2, NBS], fp8, name="pt")
                        nc.scalar.activation(pt, sp, EXP,
                                             scale=EXPSCALE, bias=ebias)
                    for j in range(2):
                        mt = 2 * mp + j
                        vsl = vaug[:, mt, h, :, :, :]
                        if diag_b:
                            rhs = pt[:, j, :][:, None, :].broadcast_to(
                                [128, 2, 256])
                            nc.tensor.matmul(
                                sap[:, 256:512], lhsT=vsl, rhs=rhs,
                                start=False, stop=(j == 1),
                                perf_mode=DR)
                        elif diag_a and j == 1:
                            rhs = pt[:, 1, 128:512][:, None, :].broadcast_to(
                                [128, 2, 384])
                            nc.tensor.matmul(
                                sap[:, 128:512], lhsT=vsl, rhs=rhs,
                                start=False, stop=False,
                                perf_mode=DR)
                        else:
                            rhs = pt[:, j, :][:, None, :].broadcast_to(
                                [128, 2, NBS])
                            nc.tensor.matmul(
                                sap, lhsT=vsl, rhs=rhs,
                                start=(mp == 0 and j == 0), stop=False,
                                perf_mode=DR)
                # normalize: denominator (broadcast 64-wide) in rows
                # 64:127, sa^T in rows 0:63.  DVE constraints (hw): recip
                # and 2-input ops need base partition 0 inputs.
                den = rrp.tile([128, NBS], f32, tag="den", name="den")
                nc.vector.tensor_copy(den[0:DH, :], sap[DH:128, :])
                rr = rrp.tile([128, NBS], f32, tag="rr", name="rr")
                nc.vector.reciprocal_approx_fast(
                    out=rr[0:DH, :], in_=den[0:DH, :])
                nc.vector.tensor_mul(
                    saT[(h % 2) * DH:(h % 2) * DH + DH, h // 2,
                        qb * NBS:(qb + 1) * NBS],
                    sap[0:DH, :], rr[0:DH, :])

            def proj(qb):
                for nt in range(4 * qb, 4 * qb + 4):
                    ot = ost.tile([128, D], bf16, name="ot")
                    for db in range(2):
                        po = mix.tile([128, 512], f32, tag="mixp",
                                      name=f"po{nt}{db}")
                        for kt in range(KT):
                            nc.tensor.matmul(
                                po,
                                lhsT=saT[:, kt, nt * 128:(nt + 1) * 128],
                                rhs=wpT[:, kt, db * 512:(db + 1) * 512],
                                start=(kt == 0), stop=(kt == KT - 1))
                        nc.vector.tensor_copy(
                            ot[:, db * 512:(db + 1) * 512], po)
                    nc.sync.dma_start(out=out_d[nt * 128:(nt + 1) * 128, :],
                                      in_=ot)

            for nt in range(4):
                v_gemm(nt)
            kq_gemm(0, 0)
            kq_gemm(2, 0)
            for qb in range(NB):
                attn(qb, 0)
                attn(qb, 1)
                kq_gemm(1, qb)
                kq_gemm(3, qb)
                attn(qb, 2)
                attn(qb, 3)
                if qb < NB - 1:
                    for nt in range(4 * qb + 4, 4 * qb + 8):
                        v_gemm(nt)
                    kq_gemm(0, qb + 1)
                    kq_gemm(2, qb + 1)
                proj(qb)

    nc.compile()
    return nc


def _host_inputs(x, Wqkv, bqkv, Wproj):
    """Per-core input maps (host-side sharding + relayout + casts)."""
    import ml_dtypes
    bf16 = ml_dtypes.bfloat16

    ident = np.eye(128, dtype=bf16)
    i = np.arange(128)[:, None]
    j = np.arange(128)[None, :]
    mneg = np.zeros((2, 128, 128), dtype=bf16)
    mneg[0] = MASKVAL
    mneg[1] = np.where(j < i, MASKVAL, 0.0).astype(bf16)

    in_maps = []
    for c in range(8):
        b, hg = c // NH, c % NH
        h0 = hg * NH
        xT = np.ascontiguousarray(x[b].T).reshape(DT, 128, N).astype(bf16)
        wq = Wqkv[h0:h0 + NH].reshape(NH, 3, DH, D)  # (h, kqv, dh, d)
        wk = wq[:, 0].reshape(2, 128, D)
        wqq = wq[:, 1].reshape(2, 128, D)
        wkq = np.stack([wk[0], wk[1], wqq[0], wqq[1]])   # [4, 128e, D]
        wkq16 = np.ascontiguousarray(
            wkq.transpose(0, 2, 1).reshape(4, DT, 128, 128)).astype(bf16)
        wv = wq[:, 2].reshape(NH * DH, D)
        wv16 = np.ascontiguousarray(wv.T.reshape(DT, 128, NH * DH)
                                    ).astype(bf16)
        bq = bqkv[h0:h0 + NH].reshape(NH, 3, DH)
        bk = bq[:, 0].reshape(2, 128)
        bqq = bq[:, 1].reshape(2, 128)
        bkq = np.stack([bk[0], bk[1], bqq[0], bqq[1]]).astype(np.float32)
        # q masks/biases: qm[0] = 0/1 partition masks per j;
        # qm[1] = masked q01 bias, qm[2] = masked q23 bias
        qm = np.zeros((3, 2, 128), dtype=np.float32)
        qm[0, 0, 0:64] = 1.0
        qm[0, 1, 64:128] = 1.0
        qm[1] = bkq[2] * qm[0]
        qm[2] = bkq[3] * qm[0]
        wpT = np.ascontiguousarray(
            Wproj[:, h0 * DH:(h0 + NH) * DH].T).astype(bf16)   # [256, D]
        in_maps.append({
            "xT": xT, "wkq": wkq16, "wv": wv16, "bkq": bkq, "qm": qm,
            "wpT": wpT, "mneg": mneg, "ident": ident,
        })
    return in_maps


def _get_nc():
    if "nc" not in _CACHE:
        _CACHE["nc"] = _build_nc()
    return _CACHE["nc"]


def run_on_hw(in_maps, trace=False, **kw):
    from concourse.bass_utils import run_bass_kernel_spmd
    nc = _get_nc()
    return run_bass_kernel_spmd(
        nc, in_maps, core_ids=list(range(8)), trace=trace, **kw)


def kernel(**inputs):
    x = np.asarray(inputs["x"], dtype=np.float32)
    Wqkv = np.asarray(inputs["Wqkv"], dtype=np.float32)
    bqkv = np.asarray(inputs["bqkv"], dtype=np.float32)
    Wproj = np.asarray(inputs["Wproj"], dtype=np.float32)
    bproj = np.asarray(inputs["bproj"], dtype=np.float32)

    in_maps = _host_inputs(x, Wqkv, bqkv, Wproj)
    res = run_on_hw(in_maps).results

    bv = bqkv.reshape(H, 3, DH)[:, 2, :].reshape(D)
    bproj_eff = bproj + Wproj @ bv

    out = np.zeros((B, N, D), dtype=np.float32)
    for b in range(B):
        acc = res[b * NH + 0]["outp"].astype(np.float32)
        for g in range(1, NH):
            acc = acc + res[b * NH + g]["outp"].astype(np.float32)
        out[b] = acc + bproj_eff[None, :]
    return out
